# revision 1
# baseline (speedup 1.0000x reference)
"""Local (sparse) attention layer on 8 Trainium2 NeuronCores.

Sharding: core c handles batch b = c//2, query half c%2 (1024 queries),
full context of its batch (data parallel on the small Dense weights).

Per-core pipeline:
  1. PE-transpose x, ctx to feature-major (needed as matmul lhsT).
  2. PE projections: q (n-major), k, v (n-major) -> packed kv rows in HBM.
  3. Per 128-query tile, per 8-neighbor quarter: indirect-DMA gather of
     packed kv rows (one row per (query, neighbor) pair), DVE dot-products
     for scores, flash-style unnormalized accumulation of exp(s)*v and
     exp(s) over neighbors (exact: scores are O(1), no max shift needed).
  4. Normalize, PE output projection, DMA out.
"""

import numpy as np

HEADS = 8
HD = 64
DIM = 512
DIN = 256
B, N, M, K = 4, 2048, 2048, 32
N_LOC = 1024  # queries per core
NT = N_LOC // 128  # query tiles per core
QK = 8  # neighbors per quarter
NQ = K // QK  # quarters

_CACHE = {}


def _build():
    import concourse.bass as bass
    import concourse.bacc as bacc
    import concourse.mybir as mybir
    from concourse.tile import TileContext
    from concourse.masks import make_identity

    f32 = mybir.dt.float32
    i32 = mybir.dt.int32

    nc = bacc.Bacc("TRN2")
    x_h = nc.dram_tensor("x", [N_LOC, DIN], f32, kind="ExternalInput")
    ctx_h = nc.dram_tensor("ctx", [M, DIN], f32, kind="ExternalInput")
    idx_h = nc.dram_tensor("idx", [128, NT * K], i32, kind="ExternalInput")
    wq_h = nc.dram_tensor("wq", [DIN, DIM], f32, kind="ExternalInput")
    wk_h = nc.dram_tensor("wk", [DIN, DIM], f32, kind="ExternalInput")
    wv_h = nc.dram_tensor("wv", [DIN, DIM], f32, kind="ExternalInput")
    wo_h = nc.dram_tensor("wo", [DIM, DIN], f32, kind="ExternalInput")
    bq_h = nc.dram_tensor("bq", [128, DIM], f32, kind="ExternalInput")
    bo_h = nc.dram_tensor("bo", [128, DIN], f32, kind="ExternalInput")
    out_h = nc.dram_tensor("out", [N_LOC, DIN], f32, kind="ExternalOutput")
    kv_h = nc.dram_tensor("kv_scratch", [M, 2 * DIM], f32, kind="Internal")

    with TileContext(nc) as tc:
        with tc.tile_pool(name="const", bufs=1) as cpool:
            ident = cpool.tile([128, 128], f32)
            make_identity(nc, ident[:])
            wq_sb = [cpool.tile([128, DIM], f32, tag=f"wq{c}", name=f"wq{c}") for c in range(2)]
            wk_sb = [cpool.tile([128, DIM], f32, tag=f"wk{c}", name=f"wk{c}") for c in range(2)]
            wv_sb = [cpool.tile([128, DIM], f32, tag=f"wv{c}", name=f"wv{c}") for c in range(2)]
            wo_sb = [cpool.tile([128, DIN], f32, tag=f"wo{c}", name=f"wo{c}") for c in range(4)]
            bq_sb = cpool.tile([128, DIM], f32)
            bo_sb = cpool.tile([128, DIN], f32)
            idx_sb = cpool.tile([128, NT * K], i32)
            for c in range(2):
                nc.sync.dma_start(out=wq_sb[c][:], in_=wq_h[c * 128:(c + 1) * 128, :])
                nc.sync.dma_start(out=wk_sb[c][:], in_=wk_h[c * 128:(c + 1) * 128, :])
                nc.sync.dma_start(out=wv_sb[c][:], in_=wv_h[c * 128:(c + 1) * 128, :])
            for c in range(4):
                nc.sync.dma_start(out=wo_sb[c][:], in_=wo_h[c * 128:(c + 1) * 128, :])
            nc.sync.dma_start(out=bq_sb[:], in_=bq_h[:])
            nc.sync.dma_start(out=bo_sb[:], in_=bo_h[:])
            nc.sync.dma_start(out=idx_sb[:], in_=idx_h[:])

            with tc.tile_pool(name="qpool", bufs=1) as qpool:
                q_sb = [qpool.tile([128, DIM], f32, tag=f"q{t}", name=f"q{t}") for t in range(NT)]

                # ---- phases 1+2: transpose inputs, project q/k/v ----
                with (
                    tc.tile_pool(name="ld", bufs=4) as ldpool,
                    tc.tile_pool(name="feat", bufs=1) as featpool,
                    tc.tile_pool(name="stage", bufs=4) as stpool,
                    tc.tile_pool(name="ps1", bufs=2, space="PSUM") as ps1,
                    tc.tile_pool(name="ps2", bufs=2, space="PSUM") as ps2,
                ):
                    x_T = [featpool.tile([128, N_LOC], f32, tag=f"xT{c}", name=f"xT{c}") for c in range(2)]
                    c_T = [featpool.tile([128, M], f32, tag=f"cT{c}", name=f"cT{c}") for c in range(2)]
                    for src_h, ntile, dst in ((ctx_h, M // 128, c_T), (x_h, NT, x_T)):
                        for t in range(ntile):
                            tile_in = ldpool.tile([128, DIN], f32, tag="ldin")
                            nc.sync.dma_start(
                                out=tile_in[:], in_=src_h[t * 128:(t + 1) * 128, :])
                            for c in range(2):
                                pst = ps1.tile([128, 128], f32, tag="tp")
                                nc.tensor.transpose(
                                    out=pst[:],
                                    in_=tile_in[:, c * 128:(c + 1) * 128],
                                    identity=ident[:])
                                nc.any.tensor_copy(
                                    out=dst[c][:, t * 128:(t + 1) * 128], in_=pst[:])
                    # k, v -> packed kv rows in HBM
                    for mt in range(M // 128):
                        psk = ps2.tile([128, DIM], f32, tag="mmk")
                        psv = ps2.tile([128, DIM], f32, tag="mmv")
                        for c in range(2):
                            nc.tensor.matmul(
                                out=psk[:], lhsT=c_T[c][:, mt * 128:(mt + 1) * 128],
                                rhs=wk_sb[c][:], start=(c == 0), stop=(c == 1))
                        for c in range(2):
                            nc.tensor.matmul(
                                out=psv[:], lhsT=c_T[c][:, mt * 128:(mt + 1) * 128],
                                rhs=wv_sb[c][:], start=(c == 0), stop=(c == 1))
                        kvt = stpool.tile([128, 2 * DIM], f32, tag="kvt")
                        nc.any.tensor_copy(out=kvt[:, :DIM], in_=psk[:])
                        nc.any.tensor_copy(out=kvt[:, DIM:], in_=psv[:])
                        nc.sync.dma_start(
                            out=kv_h[mt * 128:(mt + 1) * 128, :], in_=kvt[:])
                    # q = x @ Wq + bq  (Wq pre-scaled by 1/sqrt(hd) on host)
                    for t in range(NT):
                        psq = ps2.tile([128, DIM], f32, tag="mmq")
                        for c in range(2):
                            nc.tensor.matmul(
                                out=psq[:], lhsT=x_T[c][:, t * 128:(t + 1) * 128],
                                rhs=wq_sb[c][:], start=(c == 0), stop=(c == 1))
                        nc.vector.tensor_tensor(
                            out=q_sb[t][:], in0=psq[:],
                            in1=bq_sb[:],
                            op=mybir.AluOpType.add)

                # ---- phases 3+4: gather, attention, output projection ----
                with (
                    tc.tile_pool(name="gat", bufs=2) as gpool,
                    tc.tile_pool(name="work", bufs=2) as wpool,
                    tc.tile_pool(name="acc", bufs=2) as apool,
                    tc.tile_pool(name="ps3", bufs=4, space="PSUM") as ps3,
                    tc.tile_pool(name="ps4", bufs=2, space="PSUM") as ps4,
                ):
                    for t in range(NT):
                        av = apool.tile([128, DIM], f32, tag="av")
                        den = apool.tile([128, HEADS], f32, tag="den")
                        for qt in range(NQ):
                            kvg = gpool.tile([128, QK, 2 * DIM], f32, tag="kvg")
                            for s in range(QK):
                                col = t * K + qt * QK + s
                                nc.gpsimd.indirect_dma_start(
                                    out=kvg[:, s, :],
                                    out_offset=None,
                                    in_=kv_h[:],
                                    in_offset=bass.IndirectOffsetOnAxis(
                                        ap=idx_sb[:, col:col + 1], axis=0))
                            # scores: s[p, kk, h] = sum_d kg*q
                            prod = wpool.tile([128, QK, DIM], f32, tag="prod")
                            kg = kvg[:, :, :DIM].rearrange(
                                "p k (h d) -> p k h d", h=HEADS)
                            nc.vector.tensor_tensor(
                                out=prod[:].rearrange("p k (h d) -> p k h d", h=HEADS),
                                in0=kg,
                                in1=q_sb[t][:].rearrange(
                                    "p (o h d) -> p o h d", o=1, h=HEADS
                                ).to_broadcast([128, QK, HEADS, HD]),
                                op=mybir.AluOpType.mult)
                            sq = wpool.tile([128, QK, HEADS], f32, tag="sq")
                            nc.vector.tensor_reduce(
                                out=sq[:],
                                in_=prod[:].rearrange("p k (h d) -> p k h d", h=HEADS),
                                axis=mybir.AxisListType.X,
                                op=mybir.AluOpType.add)
                            eq = wpool.tile([128, QK, HEADS], f32, tag="eq")
                            nc.scalar.activation(
                                out=eq[:], in_=sq[:],
                                func=mybir.ActivationFunctionType.Exp)
                            # unnormalized AV accumulation
                            prod2 = wpool.tile([128, QK, DIM], f32, tag="prod")
                            vg = kvg[:, :, DIM:].rearrange(
                                "p k (h d) -> p k h d", h=HEADS)
                            nc.vector.tensor_tensor(
                                out=prod2[:].rearrange("p k (h d) -> p k h d", h=HEADS),
                                in0=vg,
                                in1=eq[:].rearrange(
                                    "p k (h o) -> p k h o", o=1
                                ).to_broadcast([128, QK, HEADS, HD]),
                                op=mybir.AluOpType.mult)
                            avp = wpool.tile([128, DIM], f32, tag="avp")
                            nc.vector.tensor_reduce(
                                out=avp[:],
                                in_=prod2[:].rearrange("p k f -> p f k"),
                                axis=mybir.AxisListType.X,
                                op=mybir.AluOpType.add)
                            dnp = wpool.tile([128, HEADS], f32, tag="dnp")
                            nc.vector.tensor_reduce(
                                out=dnp[:],
                                in_=eq[:].rearrange("p k h -> p h k"),
                                axis=mybir.AxisListType.X,
                                op=mybir.AluOpType.add)
                            if qt == 0:
                                nc.gpsimd.tensor_copy(out=av[:], in_=avp[:])
                                nc.gpsimd.tensor_copy(out=den[:], in_=dnp[:])
                            else:
                                nc.gpsimd.tensor_tensor(
                                    out=av[:], in0=av[:], in1=avp[:],
                                    op=mybir.AluOpType.add)
                                nc.gpsimd.tensor_tensor(
                                    out=den[:], in0=den[:], in1=dnp[:],
                                    op=mybir.AluOpType.add)
                        rden = apool.tile([128, HEADS], f32, tag="rden")
                        nc.vector.reciprocal(out=rden[:], in_=den[:])
                        ao = apool.tile([128, DIM], f32, tag="ao")
                        nc.vector.tensor_tensor(
                            out=ao[:].rearrange("p (h d) -> p h d", h=HEADS),
                            in0=av[:].rearrange("p (h d) -> p h d", h=HEADS),
                            in1=rden[:].rearrange(
                                "p (h o) -> p h o", o=1).to_broadcast([128, HEADS, HD]),
                            op=mybir.AluOpType.mult)
                        # transpose attnout, project to DIN
                        aT = apool.tile([128, 4 * 128], f32, tag="aT")
                        for c in range(4):
                            pst = ps3.tile([128, 128], f32, tag="tp3")
                            nc.tensor.transpose(
                                out=pst[:], in_=ao[:, c * 128:(c + 1) * 128],
                                identity=ident[:])
                            nc.any.tensor_copy(
                                out=aT[:, c * 128:(c + 1) * 128], in_=pst[:])
                        pso = ps4.tile([128, DIN], f32, tag="mo")
                        for c in range(4):
                            nc.tensor.matmul(
                                out=pso[:], lhsT=aT[:, c * 128:(c + 1) * 128],
                                rhs=wo_sb[c][:], start=(c == 0), stop=(c == 3))
                        ot = apool.tile([128, DIN], f32, tag="ot")
                        nc.vector.tensor_tensor(
                            out=ot[:], in0=pso[:],
                            in1=bo_sb[:],
                            op=mybir.AluOpType.add)
                        nc.sync.dma_start(
                            out=out_h[t * 128:(t + 1) * 128, :], in_=ot[:])
    nc.compile()
    return nc


def _get_nc():
    if "nc" not in _CACHE:
        _CACHE["nc"] = _build()
    return _CACHE["nc"]


def kernel(**inputs) -> np.ndarray:
    from concourse.bass_utils import run_bass_kernel_spmd

    x = np.asarray(inputs["x"], dtype=np.float32)
    ctx = np.asarray(inputs["context"], dtype=np.float32)
    idx = np.asarray(inputs["index_pairs"]).astype(np.int64)
    scale = 1.0 / np.sqrt(HD)
    wq = (np.asarray(inputs["Wq"], dtype=np.float32) * scale).astype(np.float32)
    bq = np.tile((np.asarray(inputs["bq"], dtype=np.float32) * scale).reshape(1, DIM), (128, 1))
    wk = np.asarray(inputs["Wk"], dtype=np.float32)
    wv = np.asarray(inputs["Wv"], dtype=np.float32)
    wo = np.asarray(inputs["Wout"], dtype=np.float32)
    bo = np.tile(np.asarray(inputs["bout"], dtype=np.float32).reshape(1, DIN), (128, 1))

    nc = _get_nc()
    in_maps = []
    for c in range(8):
        b, half = c // 2, c % 2
        x_c = np.ascontiguousarray(x[b, half * N_LOC:(half + 1) * N_LOC, :])
        idx_c = idx[b, half * N_LOC:(half + 1) * N_LOC, :]  # [1024, 32]
        # [128, NT*K]: partition = query%128, col = tile*K + kk
        idx_w = np.ascontiguousarray(
            idx_c.reshape(NT, 128, K).transpose(1, 0, 2).reshape(128, NT * K)
        ).astype(np.int32)
        in_maps.append({
            "x": x_c, "ctx": np.ascontiguousarray(ctx[b]), "idx": idx_w,
            "wq": wq, "wk": wk, "wv": wv, "wo": wo, "bq": bq, "bo": bo,
        })
    res = run_bass_kernel_spmd(nc, in_maps, core_ids=list(range(8)))
    out = np.empty((B, N, DIN), dtype=np.float32)
    for c in range(8):
        b, half = c // 2, c % 2
        out[b, half * N_LOC:(half + 1) * N_LOC, :] = res.results[c]["out"]
    return out




# revision 8
# speedup vs baseline: 1.5796x; 1.5796x over previous
"""Local (sparse) attention layer on 8 Trainium2 NeuronCores.

Sharding: core c handles batch b = c//2, query half c%2 (1024 queries),
full context of its batch (data parallel on the small Dense weights).

Per-core pipeline (fp16 on-chip, f32 accumulation where it matters):
  1. PE-transpose x, ctx to feature-major (matmul lhsT).
  2. PE projections (fp16, 1 cyc/row): q tiles in SBUF; k, v rows to HBM.
  3. Per 128-query tile: ONE indirect DMA gathers all 32 K-neighbor rows
     (4096 descriptors, 1KB each); scores via fp16 DVE mult (2x mode) +
     pairwise-halving adds split across DVE/GPSIMD; exp(s-12) on ACT
     (shift keeps fp16 weights in range; cancels in normalization).
  4. Weight broadcast over head-dim by log-doubling copies on ACT;
     V rows gathered the same way; AV = fp16 mult + halving adds;
     normalize by the weight sum; PE output projection.
"""

import numpy as np

HEADS = 8
HD = 64
DIM = 512
DIN = 256
B, N, M, K = 4, 2048, 2048, 32
N_LOC = 1024  # queries per core
NT = N_LOC // 128  # query tiles per core

_CACHE = {}


def _build():
    import concourse.bass as bass
    import concourse.bacc as bacc
    import concourse.mybir as mybir
    from concourse.tile import TileContext
    from concourse.masks import make_identity

    f16 = mybir.dt.float16
    f32 = mybir.dt.float32
    i16 = mybir.dt.int16
    ADD = mybir.AluOpType.add
    MUL = mybir.AluOpType.mult

    nc = bacc.Bacc("TRN2")
    x_h = nc.dram_tensor("x", [N_LOC, DIN], f16, kind="ExternalInput")
    ctx_h = nc.dram_tensor("ctx", [M, DIN], f16, kind="ExternalInput")
    idx_h = nc.dram_tensor("idx", [128, NT * K * 8], i16, kind="ExternalInput")
    wq_h = nc.dram_tensor("wq", [DIN, DIM], f16, kind="ExternalInput")
    wk_h = nc.dram_tensor("wk", [DIN, DIM], f16, kind="ExternalInput")
    wv_h = nc.dram_tensor("wv", [DIN, DIM], f16, kind="ExternalInput")
    wo_h = nc.dram_tensor("wo", [DIM, DIN], f16, kind="ExternalInput")
    bq_h = nc.dram_tensor("bq", [128, DIM], f32, kind="ExternalInput")
    bo_h = nc.dram_tensor("bo", [128, DIN], f32, kind="ExternalInput")
    out_h = nc.dram_tensor("out", [N_LOC, DIN], f32, kind="ExternalOutput")
    kd_h = nc.dram_tensor("k_rows", [M, DIM], f16, kind="Internal")
    vd_h = nc.dram_tensor("v_rows", [M, DIM], f16, kind="Internal")

    with TileContext(nc) as tc:
        with tc.tile_pool(name="const", bufs=1) as cpool:
            ident = cpool.tile([128, 128], f16)
            make_identity(nc, ident[:])
            wq_sb = [cpool.tile([128, DIM], f16, name=f"wq{c}") for c in range(2)]
            wk_sb = [cpool.tile([128, DIM], f16, name=f"wk{c}") for c in range(2)]
            wv_sb = [cpool.tile([128, DIM], f16, name=f"wv{c}") for c in range(2)]
            wo_sb = [cpool.tile([128, DIN], f16, name=f"wo{c}") for c in range(4)]
            bq_sb = cpool.tile([128, DIM], f32)
            bm12 = cpool.tile([128, 1], f32)
            nc.vector.memset(bm12[:], -12.0)
            bo_sb = cpool.tile([128, DIN], f32)
            idx_sb = cpool.tile([128, NT * K * 8], i16)
            for c in range(2):
                nc.sync.dma_start(out=wq_sb[c][:], in_=wq_h[c * 128:(c + 1) * 128, :])
                nc.sync.dma_start(out=wk_sb[c][:], in_=wk_h[c * 128:(c + 1) * 128, :])
                nc.sync.dma_start(out=wv_sb[c][:], in_=wv_h[c * 128:(c + 1) * 128, :])
            for c in range(4):
                nc.sync.dma_start(out=wo_sb[c][:], in_=wo_h[c * 128:(c + 1) * 128, :])
            nc.sync.dma_start(out=bq_sb[:], in_=bq_h[:])
            nc.sync.dma_start(out=bo_sb[:], in_=bo_h[:])
            nc.sync.dma_start(out=idx_sb[:], in_=idx_h[:])

            with tc.tile_pool(name="qpool", bufs=1) as qpool:
                q_sb = [qpool.tile([128, DIM], f16, name=f"q{t}") for t in range(NT)]

                # ---- phases 1+2: transpose inputs, project q/k/v ----
                with (
                    tc.tile_pool(name="ld", bufs=4) as ldpool,
                    tc.tile_pool(name="feat", bufs=1) as featpool,
                    tc.tile_pool(name="stage", bufs=4) as stpool,
                    tc.tile_pool(name="ps1", bufs=2, space="PSUM") as ps1,
                    tc.tile_pool(name="ps2", bufs=2, space="PSUM") as ps2,
                ):
                    x_T = [featpool.tile([128, N_LOC], f16, name=f"xT{c}") for c in range(2)]
                    c_T = [featpool.tile([128, M], f16, name=f"cT{c}") for c in range(2)]
                    for src_h, ntile, dst in ((ctx_h, M // 128, c_T), (x_h, NT, x_T)):
                        for t in range(ntile):
                            tile_in = ldpool.tile([128, DIN], f16, tag="ldin")
                            nc.sync.dma_start(
                                out=tile_in[:], in_=src_h[t * 128:(t + 1) * 128, :])
                            for c in range(2):
                                pst = ps1.tile([128, 128], f16, tag="tp")
                                nc.tensor.transpose(
                                    out=pst[:],
                                    in_=tile_in[:, c * 128:(c + 1) * 128],
                                    identity=ident[:])
                                nc.any.tensor_copy(
                                    out=dst[c][:, t * 128:(t + 1) * 128], in_=pst[:])
                    # k, v rows to HBM (fp16)
                    for mt in range(M // 128):
                        psk = ps2.tile([128, DIM], f32, tag="mmk")
                        psv = ps2.tile([128, DIM], f32, tag="mmv")
                        for c in range(2):
                            nc.tensor.matmul(
                                out=psk[:], lhsT=c_T[c][:, mt * 128:(mt + 1) * 128],
                                rhs=wk_sb[c][:], start=(c == 0), stop=(c == 1))
                        for c in range(2):
                            nc.tensor.matmul(
                                out=psv[:], lhsT=c_T[c][:, mt * 128:(mt + 1) * 128],
                                rhs=wv_sb[c][:], start=(c == 0), stop=(c == 1))
                        kt = stpool.tile([128, DIM], f16, tag="kt")
                        vt = stpool.tile([128, DIM], f16, tag="vt")
                        nc.any.tensor_copy(out=kt[:], in_=psk[:])
                        nc.any.tensor_copy(out=vt[:], in_=psv[:])
                        nc.sync.dma_start(
                            out=kd_h[mt * 128:(mt + 1) * 128, :], in_=kt[:])
                        nc.sync.dma_start(
                            out=vd_h[mt * 128:(mt + 1) * 128, :], in_=vt[:])
                    # q = x @ Wq + bq  (Wq, bq pre-scaled by 1/sqrt(hd) on host)
                    for t in range(NT):
                        psq = ps2.tile([128, DIM], f32, tag="mmq")
                        for c in range(2):
                            nc.tensor.matmul(
                                out=psq[:], lhsT=x_T[c][:, t * 128:(t + 1) * 128],
                                rhs=wq_sb[c][:], start=(c == 0), stop=(c == 1))
                        nc.vector.tensor_tensor(
                            out=q_sb[t][:], in0=psq[:], in1=bq_sb[:], op=ADD)

                # ---- phases 3+4: gather, attention, output projection ----
                with (
                    tc.tile_pool(name="gat", bufs=2) as gpool,
                    tc.tile_pool(name="vga", bufs=1) as vpool,
                    tc.tile_pool(name="work", bufs=1) as wpool,
                    tc.tile_pool(name="wxp", bufs=1) as wxpool,
                    tc.tile_pool(name="acc", bufs=2) as apool,
                    tc.tile_pool(name="ps3", bufs=2, space="PSUM") as ps3,
                    tc.tile_pool(name="ps4", bufs=2, space="PSUM") as ps4,
                ):
                    for t in range(NT):
                        icol = idx_sb[:, t * 8 * K:(t + 1) * 8 * K]
                        kg = gpool.tile([128, K, DIM], f16, tag="kg")
                        vg = vpool.tile([128, K, DIM], f16, tag="vg")
                        nc.gpsimd.dma_gather(
                            kg[:], kd_h[:], icol, K * 128, K * 128, DIM,
                            single_packet=False)
                        nc.gpsimd.dma_gather(
                            vg[:], vd_h[:], icol, K * 128, K * 128, DIM,
                            single_packet=False)

                        # scores: prod[p, j, h, d] = kg * q (fp16, 2x mode)
                        prod = wpool.tile([128, K * DIM], f16, tag="prod")
                        prod4 = prod[:].rearrange(
                            "p (j h d) -> p j h d", j=K, h=HEADS)
                        nc.vector.tensor_tensor(
                            out=prod4,
                            in0=kg[:].rearrange("p j (h d) -> p j h d", h=HEADS),
                            in1=q_sb[t][:].rearrange(
                                "p (o h d) -> p o h d", o=1, h=HEADS
                            ).to_broadcast([128, K, HEADS, HD]),
                            op=MUL)
                        # pairwise-halving reduce over d: 64 -> 1
                        s1 = wpool.tile([128, 8192], f16, tag="s1")
                        s2 = wpool.tile([128, 4096], f16, tag="s2")
                        lvls = [
                            (prod4, s1[:].rearrange("p (j h d) -> p j h d", j=K, h=HEADS), nc.vector),
                            (None, s2[:].rearrange("p (j h d) -> p j h d", j=K, h=HEADS), nc.vector),
                            (None, s1[:, :2048].rearrange("p (j h d) -> p j h d", j=K, h=HEADS), nc.vector),
                            (None, s2[:, :1024].rearrange("p (j h d) -> p j h d", j=K, h=HEADS), nc.vector),
                            (None, s1[:, :512].rearrange("p (j h d) -> p j h d", j=K, h=HEADS), nc.vector),
                        ]
                        cur = prod4
                        for _, dst, eng in lvls:
                            w = cur.shape[3] // 2
                            eng.tensor_tensor(
                                out=dst, in0=cur[:, :, :, 0:w], in1=cur[:, :, :, w:2 * w],
                                op=ADD)
                            cur = dst
                        s32 = apool.tile([128, K, HEADS], f32, tag="s32")
                        nc.vector.tensor_tensor(
                            out=s32[:], in0=cur[:, :, :, 0], in1=cur[:, :, :, 1], op=ADD)

                        # w = exp(s - 12) (shift cancels in normalization;
                        # keeps fp16 weights finite)
                        w32 = apool.tile([128, K, HEADS], f32, tag="w32")
                        nc.scalar.activation(
                            out=w32[:], in_=s32[:],
                            func=mybir.ActivationFunctionType.Exp, bias=bm12[:])
                        den = apool.tile([128, HEADS], f32, tag="den")
                        nc.vector.tensor_reduce(
                            out=den[:],
                            in_=w32[:].rearrange("p j h -> p h j"),
                            axis=mybir.AxisListType.X, op=ADD)

                        # broadcast w over d (to HD//2) by log-doubling on ACT
                        WX = HD // 2
                        wx = wxpool.tile([128, K, HEADS, WX], f16, tag="wx")
                        nc.scalar.activation(
                            out=wx[:, :, :, 0:1],
                            in_=w32[:].rearrange("p j (h o) -> p j h o", o=1),
                            func=mybir.ActivationFunctionType.Copy)
                        wdt = 1
                        while wdt < WX:
                            nc.scalar.activation(
                                out=wx[:, :, :, wdt:2 * wdt],
                                in_=wx[:, :, :, 0:wdt],
                                func=mybir.ActivationFunctionType.Copy)
                            wdt *= 2

                        # AV: prod re-used as two [p, j, h, 32] halves
                        vg4 = vg[:].rearrange("p j (h d) -> p j h d", h=HEADS)
                        pA = prod[:, :8192].rearrange(
                            "p (j h d) -> p j h d", j=K, h=HEADS)
                        pB = prod[:, 8192:].rearrange(
                            "p (j h d) -> p j h d", j=K, h=HEADS)
                        nc.vector.tensor_tensor(
                            out=pA, in0=vg4[:, :, :, 0:WX], in1=wx[:], op=MUL)
                        nc.vector.tensor_tensor(
                            out=pB, in0=vg4[:, :, :, WX:HD], in1=wx[:], op=MUL)
                        # pairwise-halving reduce over j: 32 -> 1
                        av32 = apool.tile([128, HEADS, HD], f32, tag="av32")
                        for half, (src, engs) in enumerate((
                                (pA, (nc.vector, nc.vector, nc.vector, nc.vector)),
                                (pB, (nc.vector, nc.vector, nc.vector, nc.vector)))):
                            cur = src
                            dsts = [
                                s1[:, :4096].rearrange("p (j h d) -> p j h d", j=16, h=HEADS),
                                s2[:, :2048].rearrange("p (j h d) -> p j h d", j=8, h=HEADS),
                                s1[:, 4096:4096 + 1024].rearrange("p (j h d) -> p j h d", j=4, h=HEADS),
                                s2[:, 2048:2048 + 512].rearrange("p (j h d) -> p j h d", j=2, h=HEADS),
                            ]
                            for dst, eng in zip(dsts, engs):
                                jw = cur.shape[1] // 2
                                eng.tensor_tensor(
                                    out=dst, in0=cur[:, 0:jw], in1=cur[:, jw:2 * jw],
                                    op=ADD)
                                cur = dst
                            nc.vector.tensor_tensor(
                                out=av32[:, :, half * WX:(half + 1) * WX],
                                in0=cur[:, 0], in1=cur[:, 1], op=ADD)

                        # normalize and project out
                        rden = apool.tile([128, HEADS], f32, tag="rden")
                        nc.vector.reciprocal(out=rden[:], in_=den[:])
                        ao = apool.tile([128, DIM], f16, tag="ao")
                        nc.vector.tensor_tensor(
                            out=ao[:].rearrange("p (h d) -> p h d", h=HEADS),
                            in0=av32[:],
                            in1=rden[:].rearrange(
                                "p (h o) -> p h o", o=1).to_broadcast([128, HEADS, HD]),
                            op=MUL)
                        aT = apool.tile([128, DIM], f16, tag="aT")
                        for c in range(4):
                            pst = ps3.tile([128, 128], f16, tag="tp3")
                            nc.tensor.transpose(
                                out=pst[:], in_=ao[:, c * 128:(c + 1) * 128],
                                identity=ident[:])
                            nc.any.tensor_copy(
                                out=aT[:, c * 128:(c + 1) * 128], in_=pst[:])
                        pso = ps4.tile([128, DIN], f32, tag="mo")
                        for c in range(4):
                            nc.tensor.matmul(
                                out=pso[:], lhsT=aT[:, c * 128:(c + 1) * 128],
                                rhs=wo_sb[c][:], start=(c == 0), stop=(c == 3))
                        ot = apool.tile([128, DIN], f32, tag="ot")
                        nc.vector.tensor_tensor(
                            out=ot[:], in0=pso[:], in1=bo_sb[:], op=ADD)
                        nc.sync.dma_start(
                            out=out_h[t * 128:(t + 1) * 128, :], in_=ot[:])
    nc.compile()
    return nc


def _get_nc():
    if "nc" not in _CACHE:
        _CACHE["nc"] = _build()
    return _CACHE["nc"]


def kernel(**inputs) -> np.ndarray:
    from concourse.bass_utils import run_bass_kernel_spmd

    x = np.asarray(inputs["x"], dtype=np.float32)
    ctx = np.asarray(inputs["context"], dtype=np.float32)
    idx = np.asarray(inputs["index_pairs"]).astype(np.int64)
    scale = 1.0 / np.sqrt(HD)
    wq = (np.asarray(inputs["Wq"], dtype=np.float32) * scale).astype(np.float16)
    bq = np.tile((np.asarray(inputs["bq"], dtype=np.float32) * scale
                  ).reshape(1, DIM), (128, 1)).astype(np.float32)
    wk = np.asarray(inputs["Wk"], dtype=np.float32).astype(np.float16)
    wv = np.asarray(inputs["Wv"], dtype=np.float32).astype(np.float16)
    wo = np.asarray(inputs["Wout"], dtype=np.float32).astype(np.float16)
    bo = np.tile(np.asarray(inputs["bout"], dtype=np.float32
                            ).reshape(1, DIN), (128, 1)).astype(np.float32)

    nc = _get_nc()
    in_maps = []
    for c in range(8):
        b, half = c // 2, c % 2
        x_c = np.ascontiguousarray(
            x[b, half * N_LOC:(half + 1) * N_LOC, :]).astype(np.float16)
        idx_c = idx[b, half * N_LOC:(half + 1) * N_LOC, :]  # [1024, 32]
        # dma_gather layout: per tile, index i = j*128 + q lives at
        # [i % 16, i // 16], replicated across the 8 groups of 16 partitions
        flat = idx_c.reshape(NT, 128, K).transpose(0, 2, 1).reshape(NT, K * 128)
        arr16 = flat.reshape(NT, K * 128 // 16, 16).transpose(2, 0, 1).reshape(
            16, NT * K * 128 // 16)
        idx_w = np.ascontiguousarray(np.tile(arr16, (8, 1))).astype(np.int16)
        in_maps.append({
            "x": x_c,
            "ctx": np.ascontiguousarray(ctx[b]).astype(np.float16),
            "idx": idx_w,
            "wq": wq, "wk": wk, "wv": wv, "wo": wo, "bq": bq, "bo": bo,
        })
    res = run_bass_kernel_spmd(nc, in_maps, core_ids=list(range(8)))
    out = np.empty((B, N, DIN), dtype=np.float32)
    for c in range(8):
        b, half = c // 2, c % 2
        out[b, half * N_LOC:(half + 1) * N_LOC, :] = res.results[c]["out"]
    return out
